# revision 2
# baseline (speedup 1.0000x reference)
"""Single-head attention Trainium2 kernel (batch=8 data-parallel over 8 cores).

Reference computation (per batch element b):
    Q = x @ Wq; K = x @ Wk; V = x @ Wv          (x: [S, D], W*: [D, O])
    out = softmax(Q @ K.T * SCALE) @ V          (SCALE = 1/8, hardcoded sqrt(64))

Kernel strategy (per core, one batch element), all-bf16 matmuls with fp32
PSUM accumulation:

  scores are reassociated:  Q K^T = x (Wq Wk^T) x^T = x M x^T
      M        = Wq Wk^T                  (bf16, 1024^3 MACs)
      T^T[d',q] = sum_d  M[d,d']  xT[d,q]   (per 512-q block)
      S^T[k,q]  = sum_d' xT[d',k] T^T[d',q]
      e         = exp(S^T / 8)            (bf16, no max-subtraction needed)
      V         = x @ Wv                  (once, in the setup phase)
      out[q,o]  = sum_ks e[ks,q] V[ks,o] / rowsum[q]

  Why bf16 and not fp8-DoubleRow: on TRN2 hardware DR is only ~1.44x over
  bf16 (and disables FWL), so a 3-term hi/lo-compensated fp8 scheme costs
  ~2x the PE time of plain bf16 for the same MACs. All matmuls here use
  512-col PSUM dsts with full 128x128 bf16 stationaries (FWL-eligible),
  which also minimizes instruction count (~2.2k matmuls/core).

  Schedule: w loads lead, x follows, wv last. PE order: wq/wk transposes,
  M, x transposes, V, then per 512-q block g: T^T(g) cols, S^T + exp,
  rowsums (tiny FWL matmuls), T^T(g+1), then attn@V with normalization on
  eviction. T^T(g+1) sits between rowsum and attn@V so its evictions
  overlap attn@V PE time and never stall S^T(g+1).
"""

import sys

sys.path.insert(0, "/opt/trn_rl_repo")

from contextlib import ExitStack

import numpy as np

import concourse.bass as bass
import concourse.mybir as mybir
from concourse import bacc
from concourse.tile import TileContext
from concourse.masks import make_identity

F32 = mybir.dt.float32
B16 = mybir.dt.bfloat16
EXP = mybir.ActivationFunctionType.Exp
SCALE = 1.0 / 8.0              # 1/sqrt(64) hardcoded by the module


def build_attn(S=2048, D=1024, O=1024, reps=1):
    """Build the Bass module for one core: x[S,D], w[3,D,O] -> out[S,O]."""
    DC = D // 128   # 8 chunks of d
    OC = O // 128   # 8 chunks of o
    KC = S // 128   # 16 chunks of s (keys)
    QB = 512        # q-block width (one full PSUM bank of fp32)
    NQB = S // QB   # 4 q-blocks
    QC = QB // 128  # 4 q-chunks per block

    nc = bacc.Bacc("TRN2", target_bir_lowering=False, debug=False)
    x_in = nc.dram_tensor("x", [S, D], F32, kind="ExternalInput")
    w_in = nc.dram_tensor("w", [3, D, O], F32, kind="ExternalInput")
    out_d = nc.dram_tensor("out", [S, O], F32, kind="ExternalOutput")

    with TileContext(nc) as tc:
      for _rep in range(reps):
        top = ExitStack()
        const_pool = top.enter_context(tc.tile_pool(name="constp", bufs=1))
        xt_pool = top.enter_context(tc.tile_pool(name="xtp", bufs=DC))
        m_pool = top.enter_context(tc.tile_pool(name="mp", bufs=DC))

        ident_f = const_pool.tile([128, 128], F32, tag="identf")
        make_identity(nc, ident_f)
        ident = const_pool.tile([128, 128], B16, tag="identb")
        nc.vector.tensor_copy(out=ident, in_=ident_f)
        ones_f = const_pool.tile([128, 1], F32, tag="onesf")
        nc.gpsimd.memset(ones_f, 1.0)
        ones_b = const_pool.tile([128, 1], B16, tag="onesb")
        nc.vector.tensor_copy(out=ones_b, in_=ones_f)

        xt = [None] * DC     # x^T  [128 d, S] bf16
        m = [None] * DC      # M    [128 d, D] bf16 (d on partitions)
        v = [None] * KC      # V    [128 s, O] bf16

        # ---------------- Phase A: w0/w1 + x, transposes, M ----------------
        with ExitStack() as ab:
            xn2_pool = ab.enter_context(tc.tile_pool(name="xn2p", bufs=KC))
            xf_pool = ab.enter_context(tc.tile_pool(name="xfp", bufs=3))
            wf_pool = ab.enter_context(tc.tile_pool(name="wfp", bufs=6))
            wn_pool = ab.enter_context(tc.tile_pool(name="wnp", bufs=10))
            wt_pool = ab.enter_context(tc.tile_pool(name="wtp", bufs=2 * OC))
            psW = ab.enter_context(tc.tile_pool(name="psW", bufs=3, space="PSUM"))
            psT = ab.enter_context(tc.tile_pool(name="psT", bufs=3, space="PSUM"))
            psM = ab.enter_context(tc.tile_pool(name="psM", bufs=2, space="PSUM"))

            wn = {0: [None] * DC, 1: [None] * DC}
            wt = {0: [None] * OC, 1: [None] * OC}
            xn2 = [None] * KC

            def load_w(j):
                for dc in range(DC):
                    wf = wf_pool.tile([128, O], F32, tag="wf", bufs=6)
                    nc.sync.dma_start(out=wf, in_=w_in[j, dc * 128:(dc + 1) * 128, :])
                    wn[j][dc] = wn_pool.tile(
                        [128, O], B16, tag="wn", bufs=10, name=f"wn{j}_{dc}"
                    )
                    if dc % 2 == 0:
                        nc.vector.tensor_copy(out=wn[j][dc], in_=wf)
                    else:
                        nc.scalar.copy(out=wn[j][dc], in_=wf)

            def transpose_w(j):
                for oc in range(OC):
                    ps = psW.tile([128, D], B16, tag="psW", bufs=3)
                    for dc in range(DC):
                        nc.tensor.transpose(
                            ps[:, dc * 128:(dc + 1) * 128],
                            wn[j][dc][:, oc * 128:(oc + 1) * 128],
                            ident,
                        )
                    wt[j][oc] = wt_pool.tile(
                        [128, D], B16, tag="wt", bufs=2 * OC, name=f"wt{j}_{oc}"
                    )
                    if oc % 2 == 0:
                        nc.vector.tensor_copy(out=wt[j][oc], in_=ps)
                    else:
                        nc.scalar.copy(out=wt[j][oc], in_=ps)

            load_w(0)
            load_w(1)
            # x DMAs (queued behind w0/w1) + casts to resident bf16 on Pool
            for kc in range(KC):
                xf = xf_pool.tile([128, D], F32, tag="xf", bufs=3)
                nc.sync.dma_start(out=xf, in_=x_in[kc * 128:(kc + 1) * 128, :])
                xn2[kc] = xn2_pool.tile(
                    [128, D], B16, tag="xn2", bufs=KC, name=f"xn2_{kc}"
                )
                nc.gpsimd.tensor_copy(out=xn2[kc], in_=xf)

            transpose_w(0)
            transpose_w(1)

            # M[d, d'] = sum_o Wq[d,o] Wk[d',o]
            for dt in range(DC):
                m[dt] = m_pool.tile([128, D], B16, tag="m", bufs=DC, name=f"m_{dt}")
                for half in range(2):
                    ps = psM.tile([128, 512], F32, tag="psM", bufs=2)
                    for oc in range(OC):
                        nc.tensor.matmul(
                            ps,
                            wt[0][oc][:, dt * 128:(dt + 1) * 128],
                            wt[1][oc][:, half * 512:(half + 1) * 512],
                            start=(oc == 0), stop=(oc == OC - 1),
                        )
                    dst = m[dt][:, half * 512:(half + 1) * 512]
                    if half == 0:
                        nc.vector.tensor_copy(out=dst, in_=ps)
                    else:
                        nc.scalar.copy(out=dst, in_=ps)

            # x transposes: xt[dc] <- x^T[d-chunk, s]
            for dc in range(DC):
                xt[dc] = xt_pool.tile(
                    [128, S], B16, tag="xt", bufs=DC, name=f"xt_{dc}"
                )
                for gp in range(KC // 8):
                    ps = psT.tile([128, 1024], B16, tag="psT", bufs=3)
                    for i in range(8):
                        kc = 8 * gp + i
                        nc.tensor.transpose(
                            ps[:, i * 128:(i + 1) * 128],
                            xn2[kc][:, dc * 128:(dc + 1) * 128],
                            ident,
                        )
                    dst = xt[dc][:, gp * 1024:(gp + 1) * 1024]
                    if (2 * dc + gp) % 2 == 0:
                        nc.vector.tensor_copy(out=dst, in_=ps)
                    else:
                        nc.scalar.copy(out=dst, in_=ps)

        # ---------------- Phase B: V = x @ Wv ----------------
        v_pool = top.enter_context(tc.tile_pool(name="vp", bufs=KC))
        with ExitStack() as vb:
            wvf_pool = vb.enter_context(tc.tile_pool(name="wvfp", bufs=3))
            wvn_pool = vb.enter_context(tc.tile_pool(name="wvnp", bufs=DC))
            psV = vb.enter_context(tc.tile_pool(name="psV", bufs=3, space="PSUM"))

            wvn = [None] * DC
            for dc in range(DC):
                wvf = wvf_pool.tile([128, O], F32, tag="wvf", bufs=3)
                nc.sync.dma_start(out=wvf, in_=w_in[2, dc * 128:(dc + 1) * 128, :])
                wvn[dc] = wvn_pool.tile(
                    [128, O], B16, tag="wvn", bufs=DC, name=f"wvn_{dc}"
                )
                if dc % 2 == 0:
                    nc.vector.tensor_copy(out=wvn[dc], in_=wvf)
                else:
                    nc.scalar.copy(out=wvn[dc], in_=wvf)

            for kc in range(KC):
                v[kc] = v_pool.tile([128, O], B16, tag="v", bufs=KC, name=f"v_{kc}")
                for oh in range(O // 512):
                    ps = psV.tile([128, 512], F32, tag="psV", bufs=3)
                    for dc in range(DC):
                        nc.tensor.matmul(
                            ps,
                            xt[dc][:, kc * 128:(kc + 1) * 128],
                            wvn[dc][:, oh * 512:(oh + 1) * 512],
                            start=(dc == 0), stop=(dc == DC - 1),
                        )
                    dst = v[kc][:, oh * 512:(oh + 1) * 512]
                    if (2 * kc + oh) % 2 == 0:
                        nc.vector.tensor_copy(out=dst, in_=ps)
                    else:
                        nc.scalar.copy(out=dst, in_=ps)

        # ---------------- Phase C: per 512-q block ----------------
        tt_pool = top.enter_context(tc.tile_pool(name="ttp", bufs=DC))
        tt = [None] * DC     # T^T [128 d', S] bf16

        with ExitStack() as ph_c:
            eb_pool = ph_c.enter_context(tc.tile_pool(name="ebp", bufs=KC + 2))
            outs_pool = ph_c.enter_context(tc.tile_pool(name="outsp", bufs=4))
            small_pool = ph_c.enter_context(tc.tile_pool(name="smallp", bufs=3))
            pcs = ph_c.enter_context(tc.tile_pool(name="pcs", bufs=4, space="PSUM"))
            pcsum = ph_c.enter_context(tc.tile_pool(name="pcsum", bufs=1, space="PSUM"))
            pco = ph_c.enter_context(tc.tile_pool(name="pco", bufs=3, space="PSUM"))

            def emit_tt_sb(g):
                """T^T columns g*512..(g+1)*512 for all 8 d'-chunks."""
                for pc in range(DC):
                    if g == 0:
                        tt[pc] = tt_pool.tile(
                            [128, S], B16, tag="tt", bufs=DC, name=f"tt_{pc}"
                        )
                    ps = pcs.tile([128, QB], F32, tag="pcs", bufs=4)
                    for dc in range(DC):
                        nc.tensor.matmul(
                            ps,
                            m[dc][:, pc * 128:(pc + 1) * 128],
                            xt[dc][:, g * QB:(g + 1) * QB],
                            start=(dc == 0), stop=(dc == DC - 1),
                        )
                    dst = tt[pc][:, g * QB:(g + 1) * QB]
                    if pc % 2 == 0:
                        nc.vector.tensor_copy(out=dst, in_=ps)
                    else:
                        nc.scalar.copy(out=dst, in_=ps)

            def emit_qb(g, last):
                q0 = g * QB
                # scores^T[k, q] per kc; one exp per kc -> eb bf16
                ebs = []
                for kc in range(KC):
                    ps_s = pcs.tile([128, QB], F32, tag="pcs", bufs=4)
                    for pc in range(DC):
                        nc.tensor.matmul(
                            ps_s,
                            xt[pc][:, kc * 128:(kc + 1) * 128],
                            tt[pc][:, q0:q0 + QB],
                            start=(pc == 0), stop=(pc == DC - 1),
                        )
                    eb = eb_pool.tile([128, QB], B16, tag="eb", bufs=KC + 2)
                    nc.scalar.activation(out=eb, in_=ps_s, func=EXP, scale=SCALE)
                    ebs.append(eb)
                # row sums: tiny FWL matmuls, 16 per q-chunk, then reciprocal
                ps_sum = pcsum.tile([128, QC], F32, tag="pcsum", bufs=1)
                for qc in range(QC):
                    for kc in range(KC):
                        nc.tensor.matmul(
                            ps_sum[:, qc:qc + 1],
                            ebs[kc][:, qc * 128:(qc + 1) * 128],
                            ones_b,
                            start=(kc == 0), stop=(kc == KC - 1),
                        )
                rc = small_pool.tile([128, QC], F32, tag="rc", bufs=3)
                nc.vector.reciprocal(out=rc, in_=ps_sum)
                # T^T for the next block overlaps attn@V below
                if not last:
                    emit_tt_sb(g + 1)
                # out[q, o] = (e^T V) * rc
                for qc in range(QC):
                    for oh in range(O // 512):
                        ps_o = pco.tile([128, 512], F32, tag="pco", bufs=3)
                        for kc in range(KC):
                            nc.tensor.matmul(
                                ps_o,
                                ebs[kc][:, qc * 128:(qc + 1) * 128],
                                v[kc][:, oh * 512:(oh + 1) * 512],
                                start=(kc == 0), stop=(kc == KC - 1),
                            )
                        os_ = outs_pool.tile([128, 512], F32, tag="outs", bufs=4)
                        nc.vector.tensor_scalar_mul(
                            out=os_, in0=ps_o, scalar1=rc[:, qc:qc + 1]
                        )
                        nc.sync.dma_start(
                            out=out_d[
                                q0 + qc * 128:q0 + (qc + 1) * 128,
                                oh * 512:(oh + 1) * 512,
                            ],
                            in_=os_,
                        )

            emit_tt_sb(0)
            for g in range(NQB):
                emit_qb(g, last=(g == NQB - 1))

        top.close()

    nc.compile()
    return nc


_NC_CACHE = {}


def _get_nc():
    key = "full"
    if key not in _NC_CACHE:
        _NC_CACHE[key] = build_attn()
    return _NC_CACHE[key]


def kernel(**inputs):
    """Full-input entry point: x [8, 2048, 1024], kernel [3, 1024, 1024]."""
    from concourse.bass_utils import run_bass_kernel_spmd

    x = np.ascontiguousarray(inputs["x"], dtype=np.float32)
    w = np.ascontiguousarray(inputs["kernel"], dtype=np.float32)
    B = x.shape[0]
    nc = _get_nc()
    in_maps = [{"x": x[b], "w": w} for b in range(B)]
    res = run_bass_kernel_spmd(nc, in_maps, core_ids=list(range(B)))
    return np.stack([res.results[b]["out"] for b in range(B)], axis=0)
